# revision 2
# baseline (speedup 1.0000x reference)
"""Trainium2 Bass kernel for nn_LogicLayer (difflogic LogicLayer forward).

Computation (reference):
    w  = softmax(weights, axis=-1)            # [OUT, 16]
    c  = w @ GATE_M                           # [OUT, 4]
    a  = x[:, idx_a]; b = x[:, idx_b]         # [B, OUT] feature gathers
    out = c0 + c1*a + c2*b + c3*(a*b)

Strategy (8 NeuronCores, data-parallel over batch):
  - Each core gets a batch shard of 512 rows, uploaded pre-transposed as
    xT [IN, 512] so the feature gather becomes a contiguous-row gather.
  - idx_a / idx_b are baked into int16 SWDGE gather index buffers
    (dma_gather: each index pulls one 2 KB row of xT from HBM into SBUF,
    landing at partition i%128).
  - Gate coefficients c0..c3 are computed on-device from `weights`
    (exp on ScalarE, strided-AP reductions + small tensor ops on VectorE).
  - Fused multiply-add via tensor_scalar / scalar_tensor_tensor with
    per-partition coefficient scalars.
  - Output is written transposed (outT [OUT, 512]) so DMA writes are
    contiguous 2 KB runs; the host unshard transposes back.
"""

import os
import numpy as np

BATCH, IN_DIM, OUT_DIM = 4096, 16384, 16384
N_CORES = 8
B_CORE = BATCH // N_CORES  # 512
P = 128


def _build_nc(in_dim, out_dim, b_core, chunk=1024):
    """Build + compile the per-core Bass program (SPMD, identical on all cores)."""
    from contextlib import ExitStack

    import concourse.bacc as bacc
    import concourse.mybir as mybir
    import concourse.tile as tile

    F32 = mybir.dt.float32
    I16 = mybir.dt.int16
    TT = out_dim // P       # total coefficient columns (j = tt*128 + p)
    TC = chunk // P         # t-slices per chunk
    NCH = out_dim // chunk  # number of gather/compute chunks
    mult = mybir.AluOpType.mult
    add = mybir.AluOpType.add
    subtract = mybir.AluOpType.subtract

    nc = bacc.Bacc("TRN2", target_bir_lowering=False, debug=False)
    xT = nc.dram_tensor("xT", [in_dim, b_core], F32, kind="ExternalInput")
    w = nc.dram_tensor("w", [out_dim, 16], F32, kind="ExternalInput")
    ia = nc.dram_tensor("ia", [P, out_dim // 16], I16, kind="ExternalInput")
    ib = nc.dram_tensor("ib", [P, out_dim // 16], I16, kind="ExternalInput")
    outT = nc.dram_tensor("outT", [out_dim, b_core], F32, kind="ExternalOutput")

    with tile.TileContext(nc) as tc, ExitStack() as ctx:
        const_pool = ctx.enter_context(tc.tile_pool(name="const", bufs=1))
        setup_pool = ctx.enter_context(tc.tile_pool(name="setup", bufs=1))
        ab_pool = ctx.enter_context(tc.tile_pool(name="ab", bufs=2))
        m_pool = ctx.enter_context(tc.tile_pool(name="m", bufs=2))
        s_pool = ctx.enter_context(tc.tile_pool(name="s", bufs=2))
        sc_pool = ctx.enter_context(tc.tile_pool(name="sc", bufs=4))

        # ---------- index buffers ----------
        ia_sb = const_pool.tile([P, out_dim // 16], I16, tag="ia")
        ib_sb = const_pool.tile([P, out_dim // 16], I16, tag="ib")
        nc.sync.dma_start(ia_sb[:], ia[:])
        nc.sync.dma_start(ib_sb[:], ib[:])

        # ---------- gate coefficients ----------
        # w_sb[p, tt, g] = weights[tt*128 + p, g]
        w_sb = setup_pool.tile([P, TT, 16], F32, tag="wsb")
        nc.sync.dma_start(w_sb[:], w[:].rearrange("(t p) g -> p t g", p=P))
        E = setup_pool.tile([P, TT, 16], F32, tag="E")
        nc.scalar.activation(E[:], w_sb[:], mybir.ActivationFunctionType.Exp)

        su = setup_pool.tile([P, TT], F32, tag="su")
        nc.vector.reduce_sum(su[:], E[:], axis=mybir.AxisListType.X)
        r = setup_pool.tile([P, TT], F32, tag="r")
        nc.vector.reciprocal(r[:], su[:])

        # unnormalized coefficient columns of E @ GATE_M
        # gate matrix rows g -> [c0,c1,c2,c3]; see reference _GATE_M
        c0u = setup_pool.tile([P, TT], F32, tag="c0u")
        nc.vector.reduce_sum(c0u[:], E[:, :, 8:16], axis=mybir.AxisListType.X)

        E4 = E[:].rearrange("p t (g2 g1) -> p t g2 g1", g1=4)
        a1 = setup_pool.tile([P, TT], F32, tag="a1")
        nc.vector.reduce_sum(a1[:], E4[:, :, 0:2, 2:4], axis=mybir.AxisListType.XY)
        b1 = setup_pool.tile([P, TT], F32, tag="b1")
        nc.vector.reduce_sum(b1[:], E4[:, :, 2:4, 0:2], axis=mybir.AxisListType.XY)
        c1u = setup_pool.tile([P, TT], F32, tag="c1u")
        nc.vector.tensor_tensor(c1u[:], a1[:], b1[:], op=subtract)

        a2 = setup_pool.tile([P, TT], F32, tag="a2")
        nc.vector.reduce_sum(a2[:], E[:, :, 4:8], axis=mybir.AxisListType.X)
        b2 = setup_pool.tile([P, TT], F32, tag="b2")
        nc.vector.reduce_sum(b2[:], E[:, :, 8:12], axis=mybir.AxisListType.X)
        c2u = setup_pool.tile([P, TT], F32, tag="c2u")
        nc.vector.tensor_tensor(c2u[:], a2[:], b2[:], op=subtract)

        # c3 = E1 - E2 - E4 - 2*E6 - E7 + E8 + 2*E9 + E11 + E13 - E14
        #    = (E1+E8) + (E11+E13) - (E2+E4) - (E7+E14) - 2*(E6-E9)
        def eg(g):
            return E[:, :, g : g + 1]

        p1 = setup_pool.tile([P, TT, 1], F32, tag="p1")
        nc.vector.tensor_tensor(p1[:], eg(1), eg(8), op=add)
        p2 = setup_pool.tile([P, TT, 1], F32, tag="p2")
        nc.vector.tensor_tensor(p2[:], eg(11), eg(13), op=add)
        n1 = setup_pool.tile([P, TT, 1], F32, tag="n1")
        nc.vector.tensor_tensor(n1[:], eg(2), eg(4), op=add)
        n2 = setup_pool.tile([P, TT, 1], F32, tag="n2")
        nc.vector.tensor_tensor(n2[:], eg(7), eg(14), op=add)
        d6 = setup_pool.tile([P, TT, 1], F32, tag="d6")
        nc.vector.tensor_tensor(d6[:], eg(6), eg(9), op=subtract)
        pp = setup_pool.tile([P, TT, 1], F32, tag="pp")
        nc.vector.tensor_tensor(pp[:], p1[:], p2[:], op=add)
        nn_ = setup_pool.tile([P, TT, 1], F32, tag="nn")
        nc.vector.tensor_tensor(nn_[:], n1[:], n2[:], op=add)
        c3a = setup_pool.tile([P, TT, 1], F32, tag="c3a")
        nc.vector.tensor_tensor(c3a[:], pp[:], nn_[:], op=subtract)
        c3u = setup_pool.tile([P, TT, 1], F32, tag="c3u")
        # c3u = (d6 * -2) + c3a
        nc.vector.scalar_tensor_tensor(
            c3u[:], d6[:], -2.0, c3a[:], op0=mult, op1=add
        )

        # normalized coefficients (softmax denominator)
        c0 = const_pool.tile([P, TT], F32, tag="c0")
        nc.vector.tensor_tensor(c0[:], c0u[:], r[:], op=mult)
        c1 = const_pool.tile([P, TT], F32, tag="c1")
        nc.vector.tensor_tensor(c1[:], c1u[:], r[:], op=mult)
        c2 = const_pool.tile([P, TT], F32, tag="c2")
        nc.vector.tensor_tensor(c2[:], c2u[:], r[:], op=mult)
        c3 = const_pool.tile([P, TT], F32, tag="c3")
        nc.vector.tensor_tensor(c3[:], c3u[:, :, 0], r[:], op=mult)

        # ---------- main gather + FMA loop ----------
        outT_r = outT[:].rearrange("(c t p) e -> c p t e", t=TC, p=P)
        idx_cols = chunk // 16
        for ci in range(NCH):
            a_t = ab_pool.tile([P, TC, b_core], F32, tag="a")
            b_t = ab_pool.tile([P, TC, b_core], F32, tag="b")
            nc.gpsimd.dma_gather(
                a_t[:], xT[:], ia_sb[:, ci * idx_cols : (ci + 1) * idx_cols],
                chunk, chunk, b_core,
            )
            nc.gpsimd.dma_gather(
                b_t[:], xT[:], ib_sb[:, ci * idx_cols : (ci + 1) * idx_cols],
                chunk, chunk, b_core,
            )
            m_t = m_pool.tile([P, TC, b_core], F32, tag="m")
            nc.vector.tensor_tensor(m_t[:], a_t[:], b_t[:], op=mult)
            s_t = s_pool.tile([P, TC, b_core], F32, tag="s")
            for t in range(TC):
                tt = ci * TC + t
                u = sc_pool.tile([P, b_core], F32, tag="u")
                # u = a*c1 + c0
                nc.any.tensor_scalar(
                    u[:], a_t[:, t, :], c1[:, tt : tt + 1], c0[:, tt : tt + 1],
                    op0=mult, op1=add,
                )
                v = sc_pool.tile([P, b_core], F32, tag="v")
                # v = b*c2 + u
                nc.vector.scalar_tensor_tensor(
                    v[:], b_t[:, t, :], c2[:, tt : tt + 1], u[:], op0=mult, op1=add
                )
                # s = m*c3 + v
                nc.vector.scalar_tensor_tensor(
                    s_t[:, t, :], m_t[:, t, :], c3[:, tt : tt + 1], v[:],
                    op0=mult, op1=add,
                )
            nc.sync.dma_start(outT_r[ci], s_t[:])

    nc.compile()
    return nc


def _wrap_idx16(idx, chunk=1024):
    """Host-side: build the [128, OUT//16] int16 SWDGE gather index buffer.

    Within each chunk of `chunk` indices, dma_gather consumes index number
    i from partition i%16, column i//16 (replicated across the 8 groups of
    16 partitions).
    """
    idx = np.asarray(idx).astype(np.int16)
    out_dim = idx.shape[0]
    blocks = []
    for c in range(out_dim // chunk):
        blk = idx[c * chunk : (c + 1) * chunk].reshape(chunk // 16, 16)
        blocks.append(np.tile(blk.T, (P // 16, 1)))  # [128, chunk//16]
    return np.ascontiguousarray(np.concatenate(blocks, axis=1))


_NC_CACHE = {}


def _get_nc():
    key = (IN_DIM, OUT_DIM, B_CORE)
    if key not in _NC_CACHE:
        _NC_CACHE[key] = _build_nc(IN_DIM, OUT_DIM, B_CORE)
    return _NC_CACHE[key]


TRACE = False  # set by dev harness to capture an NTFF profile
LAST_RESULT = None


def kernel(x, weights, idx_a, idx_b):
    global LAST_RESULT
    from concourse.bass_utils import run_bass_kernel_spmd

    x = np.asarray(x, dtype=np.float32)
    weights = np.asarray(weights, dtype=np.float32)
    ia16 = _wrap_idx16(idx_a)
    ib16 = _wrap_idx16(idx_b)

    nc = _get_nc()
    in_maps = []
    for k in range(N_CORES):
        xT_k = np.ascontiguousarray(x[k * B_CORE : (k + 1) * B_CORE, :].T)
        in_maps.append({"xT": xT_k, "w": weights, "ia": ia16, "ib": ib16})

    res = run_bass_kernel_spmd(
        nc, in_maps, list(range(N_CORES)), trace=TRACE
    )
    LAST_RESULT = res
    out = np.empty((BATCH, OUT_DIM), dtype=np.float32)
    for k in range(N_CORES):
        out[k * B_CORE : (k + 1) * B_CORE, :] = res.results[k]["outT"].T
    return out


# revision 3
# speedup vs baseline: 1.4001x; 1.4001x over previous
"""Trainium2 Bass kernel for nn_LogicLayer (difflogic LogicLayer forward).

Computation (reference):
    w  = softmax(weights, axis=-1)            # [OUT, 16]
    c  = w @ GATE_M                           # [OUT, 4]
    a  = x[:, idx_a]; b = x[:, idx_b]         # [B, OUT] feature gathers
    out = c0 + c1*a + c2*b + c3*(a*b)

Strategy (8 NeuronCores, feature-parallel):
  - x is uploaded transposed (xT [IN, B]) and replicated; each core
    computes OUT/8 = 2048 output features over the full batch.
  - Per output feature, dma_gather pulls the two needed xT rows (16 KB
    each) from HBM by int16 index — one descriptor per row, so SWDGE
    descriptor generation (~12 ns/desc on the Q7) stays tiny.
  - Gate coefficients c0..c3 are computed on-device from `weights`
    (exp on ScalarE, strided-AP reductions + small tensor ops on VectorE).
  - out = (c0 + c1*a) + b*(c2 + c3*a): the two parenthesized terms are
    per-partition-scalar affine maps of `a` (ScalarE Identity
    activation), combined by two VectorE tensor_tensor passes.
  - Output written as outT [2048, B] (contiguous 16 KB per partition);
    host unshard transposes back.
"""

import numpy as np

BATCH, IN_DIM, OUT_DIM = 4096, 16384, 16384
N_CORES = 8
F_CORE = OUT_DIM // N_CORES  # 2048 output features per core
P = 128


def _build_nc(in_dim, feat_core, batch):
    """Build + compile the per-core Bass program (SPMD, identical cores)."""
    from contextlib import ExitStack

    import concourse.bacc as bacc
    import concourse.mybir as mybir
    import concourse.tile as tile

    F32 = mybir.dt.float32
    I16 = mybir.dt.int16
    TT = feat_core // P  # feature chunks per core (16)
    mult = mybir.AluOpType.mult
    add = mybir.AluOpType.add
    subtract = mybir.AluOpType.subtract
    Ident = mybir.ActivationFunctionType.Identity

    nc = bacc.Bacc("TRN2", target_bir_lowering=False, debug=False)
    xT = nc.dram_tensor("xT", [in_dim, batch], F32, kind="ExternalInput")
    w = nc.dram_tensor("w", [feat_core, 16], F32, kind="ExternalInput")
    # combined gather indices: per chunk, 128 idx_a then 128 idx_b
    idx = nc.dram_tensor("idx", [P, 2 * feat_core // 16], I16, kind="ExternalInput")
    outT = nc.dram_tensor("outT", [feat_core, batch], F32, kind="ExternalOutput")

    with tile.TileContext(nc) as tc, ExitStack() as ctx:
        const_pool = ctx.enter_context(tc.tile_pool(name="const", bufs=1))
        g_pool = ctx.enter_context(tc.tile_pool(name="g", bufs=2))
        uv_pool = ctx.enter_context(tc.tile_pool(name="uv", bufs=2))
        s_pool = ctx.enter_context(tc.tile_pool(name="s", bufs=2))

        idx_sb = const_pool.tile([P, 2 * feat_core // 16], I16, tag="idx")
        nc.sync.dma_start(idx_sb[:], idx[:])

        c0 = const_pool.tile([P, TT], F32, tag="c0")
        c1 = const_pool.tile([P, TT], F32, tag="c1")
        c2 = const_pool.tile([P, TT], F32, tag="c2")
        c3 = const_pool.tile([P, TT], F32, tag="c3")

        # ---------- gate coefficients (small setup, freed after) ----------
        with tc.tile_pool(name="setup", bufs=1) as sp:
            w_sb = sp.tile([P, TT, 16], F32, tag="wsb")
            nc.sync.dma_start(w_sb[:], w[:].rearrange("(t p) g -> p t g", p=P))
            E = sp.tile([P, TT, 16], F32, tag="E")
            nc.scalar.activation(E[:], w_sb[:], mybir.ActivationFunctionType.Exp)

            su = sp.tile([P, TT], F32, tag="su")
            nc.vector.reduce_sum(su[:], E[:], axis=mybir.AxisListType.X)
            r = sp.tile([P, TT], F32, tag="r")
            nc.vector.reciprocal(r[:], su[:])

            c0u = sp.tile([P, TT], F32, tag="c0u")
            nc.vector.reduce_sum(c0u[:], E[:, :, 8:16], axis=mybir.AxisListType.X)

            E4 = E[:].rearrange("p t (g2 g1) -> p t g2 g1", g1=4)
            a1 = sp.tile([P, TT], F32, tag="a1")
            nc.vector.reduce_sum(a1[:], E4[:, :, 0:2, 2:4], axis=mybir.AxisListType.XY)
            b1 = sp.tile([P, TT], F32, tag="b1")
            nc.vector.reduce_sum(b1[:], E4[:, :, 2:4, 0:2], axis=mybir.AxisListType.XY)
            c1u = sp.tile([P, TT], F32, tag="c1u")
            nc.vector.tensor_tensor(c1u[:], a1[:], b1[:], op=subtract)

            a2 = sp.tile([P, TT], F32, tag="a2")
            nc.vector.reduce_sum(a2[:], E[:, :, 4:8], axis=mybir.AxisListType.X)
            b2 = sp.tile([P, TT], F32, tag="b2")
            nc.vector.reduce_sum(b2[:], E[:, :, 8:12], axis=mybir.AxisListType.X)
            c2u = sp.tile([P, TT], F32, tag="c2u")
            nc.vector.tensor_tensor(c2u[:], a2[:], b2[:], op=subtract)

            # c3 = (E1+E8) + (E11+E13) - (E2+E4) - (E7+E14) - 2*(E6-E9)
            def eg(g):
                return E[:, :, g : g + 1]

            p1 = sp.tile([P, TT, 1], F32, tag="p1")
            nc.vector.tensor_tensor(p1[:], eg(1), eg(8), op=add)
            p2 = sp.tile([P, TT, 1], F32, tag="p2")
            nc.vector.tensor_tensor(p2[:], eg(11), eg(13), op=add)
            n1 = sp.tile([P, TT, 1], F32, tag="n1")
            nc.vector.tensor_tensor(n1[:], eg(2), eg(4), op=add)
            n2 = sp.tile([P, TT, 1], F32, tag="n2")
            nc.vector.tensor_tensor(n2[:], eg(7), eg(14), op=add)
            d6 = sp.tile([P, TT, 1], F32, tag="d6")
            nc.vector.tensor_tensor(d6[:], eg(6), eg(9), op=subtract)
            pp = sp.tile([P, TT, 1], F32, tag="pp")
            nc.vector.tensor_tensor(pp[:], p1[:], p2[:], op=add)
            nn_ = sp.tile([P, TT, 1], F32, tag="nn")
            nc.vector.tensor_tensor(nn_[:], n1[:], n2[:], op=add)
            c3a = sp.tile([P, TT, 1], F32, tag="c3a")
            nc.vector.tensor_tensor(c3a[:], pp[:], nn_[:], op=subtract)
            c3u = sp.tile([P, TT, 1], F32, tag="c3u")
            nc.vector.scalar_tensor_tensor(
                c3u[:], d6[:], -2.0, c3a[:], op0=mult, op1=add
            )

            nc.vector.tensor_tensor(c0[:], c0u[:], r[:], op=mult)
            nc.vector.tensor_tensor(c1[:], c1u[:], r[:], op=mult)
            nc.vector.tensor_tensor(c2[:], c2u[:], r[:], op=mult)
            nc.vector.tensor_tensor(c3[:], c3u[:, :, 0], r[:], op=mult)

        # ---------- main gather + FMA loop ----------
        for ci in range(TT):
            g_t = g_pool.tile([P, 2, batch], F32, tag="g")
            nc.gpsimd.dma_gather(
                g_t[:], xT[:], idx_sb[:, ci * 16 : (ci + 1) * 16], 256, 256, batch
            )
            a_v = g_t[:, 0, :]
            b_v = g_t[:, 1, :]
            cs = slice(ci, ci + 1)
            # u = c0 + c1*a ; v = c2 + c3*a   (ScalarE, per-partition affine)
            u = uv_pool.tile([P, batch], F32, tag="u")
            nc.scalar.activation(u[:], a_v, Ident, bias=c0[:, cs], scale=c1[:, cs])
            v = uv_pool.tile([P, batch], F32, tag="v")
            nc.scalar.activation(v[:], a_v, Ident, bias=c2[:, cs], scale=c3[:, cs])
            # s = v*b + u  (VectorE)
            s_t = s_pool.tile([P, batch], F32, tag="s")
            nc.vector.tensor_tensor(s_t[:], v[:], b_v, op=mult)
            nc.vector.tensor_tensor(s_t[:], s_t[:], u[:], op=add)
            nc.sync.dma_start(outT[ci * P : (ci + 1) * P, :], s_t[:])

    nc.compile()
    return nc


def _pack_idx(idx_a, idx_b, feat_lo, feat_hi):
    """Host-side int16 gather-index buffer for one core.

    Per 128-feature chunk: 128 idx_a then 128 idx_b. dma_gather consumes
    index i from partition i%16, column i//16 (replicated across the 8
    groups of 16 partitions).
    """
    cols = []
    for f0 in range(feat_lo, feat_hi, P):
        ids = np.concatenate(
            [idx_a[f0 : f0 + P], idx_b[f0 : f0 + P]]
        ).astype(np.int16)
        blk = ids.reshape(16, 16)  # [col, partition-within-16]
        cols.append(np.tile(blk.T, (P // 16, 1)))  # [128, 16]
    return np.ascontiguousarray(np.concatenate(cols, axis=1))


_NC_CACHE = {}


def _get_nc():
    key = (IN_DIM, F_CORE, BATCH)
    if key not in _NC_CACHE:
        _NC_CACHE[key] = _build_nc(IN_DIM, F_CORE, BATCH)
    return _NC_CACHE[key]


TRACE = False  # set by dev harness to capture an NTFF profile
LAST_RESULT = None


def kernel(x, weights, idx_a, idx_b):
    global LAST_RESULT
    from concourse.bass_utils import run_bass_kernel_spmd

    x = np.asarray(x, dtype=np.float32)
    weights = np.asarray(weights, dtype=np.float32)
    idx_a = np.asarray(idx_a)
    idx_b = np.asarray(idx_b)

    nc = _get_nc()
    xT = np.ascontiguousarray(x.T)
    in_maps = []
    for k in range(N_CORES):
        lo, hi = k * F_CORE, (k + 1) * F_CORE
        in_maps.append(
            {
                "xT": xT,
                "w": np.ascontiguousarray(weights[lo:hi]),
                "idx": _pack_idx(idx_a, idx_b, lo, hi),
            }
        )

    res = run_bass_kernel_spmd(nc, in_maps, list(range(N_CORES)), trace=TRACE)
    LAST_RESULT = res
    out = np.empty((BATCH, OUT_DIM), dtype=np.float32)
    for k in range(N_CORES):
        out[:, k * F_CORE : (k + 1) * F_CORE] = res.results[k]["outT"].T
    return out


# revision 5
# speedup vs baseline: 1.5319x; 1.0941x over previous
"""Trainium2 Bass kernel for nn_LogicLayer (difflogic LogicLayer forward).

Computation (reference):
    w  = softmax(weights, axis=-1)            # [OUT, 16]
    c  = w @ GATE_M                           # [OUT, 4]
    a  = x[:, idx_a]; b = x[:, idx_b]         # [B, OUT] feature gathers
    out = c0 + c1*a + c2*b + c3*(a*b)

Strategy (8 NeuronCores, feature-parallel):
  - x is uploaded transposed (xT [IN, B]) and replicated; each core
    computes OUT/8 = 2048 output features over the full batch.
  - Per output feature, dma_gather pulls the two needed xT rows (16 KB
    each) from HBM by int16 index — one descriptor per row, so SWDGE
    descriptor generation (~12 ns/desc on the Q7) stays tiny.
  - Gate coefficients c0..c3 are computed on-device from `weights`
    (exp on ScalarE, strided-AP reductions + small tensor ops on VectorE).
  - out = (c0 + c1*a) + b*(c2 + c3*a): the two parenthesized terms are
    per-partition-scalar affine maps of `a` (ScalarE Identity
    activation), combined by two VectorE tensor_tensor passes.
  - Output written as outT [2048, B] (contiguous 16 KB per partition);
    host unshard transposes back.
"""

import numpy as np

BATCH, IN_DIM, OUT_DIM = 4096, 16384, 16384
N_CORES = 8
F_CORE = OUT_DIM // N_CORES  # 2048 output features per core
P = 128


def _build_nc(in_dim, feat_core, batch):
    """Build + compile the per-core Bass program (SPMD, identical cores)."""
    from contextlib import ExitStack

    import concourse.bacc as bacc
    import concourse.mybir as mybir
    import concourse.tile as tile

    F32 = mybir.dt.float32
    I16 = mybir.dt.int16
    TT = feat_core // P  # feature chunks per core (16)
    mult = mybir.AluOpType.mult
    add = mybir.AluOpType.add
    subtract = mybir.AluOpType.subtract
    Ident = mybir.ActivationFunctionType.Identity

    nc = bacc.Bacc("TRN2", target_bir_lowering=False, debug=False)
    xT = nc.dram_tensor("xT", [in_dim, batch], F32, kind="ExternalInput")
    w = nc.dram_tensor("w", [feat_core, 16], F32, kind="ExternalInput")
    # combined gather indices: per chunk, 128 idx_a then 128 idx_b
    idx = nc.dram_tensor("idx", [P, 2 * feat_core // 16], I16, kind="ExternalInput")
    outT = nc.dram_tensor("outT", [feat_core, batch], F32, kind="ExternalOutput")

    with tile.TileContext(nc) as tc, ExitStack() as ctx:
        const_pool = ctx.enter_context(tc.tile_pool(name="const", bufs=1))
        g_pool = ctx.enter_context(tc.tile_pool(name="g", bufs=3))
        uv_pool = ctx.enter_context(tc.tile_pool(name="uv", bufs=2))

        idx_sb = const_pool.tile([P, 2 * feat_core // 16], I16, tag="idx")
        nc.sync.dma_start(idx_sb[:], idx[:])

        c0 = const_pool.tile([P, TT], F32, tag="c0")
        c1 = const_pool.tile([P, TT], F32, tag="c1")
        c2 = const_pool.tile([P, TT], F32, tag="c2")
        c3 = const_pool.tile([P, TT], F32, tag="c3")

        # ---------- gate coefficients (small setup, freed after) ----------
        with tc.tile_pool(name="setup", bufs=1) as sp:
            w_sb = sp.tile([P, TT, 16], F32, tag="wsb")
            nc.sync.dma_start(w_sb[:], w[:].rearrange("(t p) g -> p t g", p=P))
            E = sp.tile([P, TT, 16], F32, tag="E")
            nc.scalar.activation(E[:], w_sb[:], mybir.ActivationFunctionType.Exp)

            su = sp.tile([P, TT], F32, tag="su")
            nc.vector.reduce_sum(su[:], E[:], axis=mybir.AxisListType.X)
            r = sp.tile([P, TT], F32, tag="r")
            nc.vector.reciprocal(r[:], su[:])

            c0u = sp.tile([P, TT], F32, tag="c0u")
            nc.vector.reduce_sum(c0u[:], E[:, :, 8:16], axis=mybir.AxisListType.X)

            E4 = E[:].rearrange("p t (g2 g1) -> p t g2 g1", g1=4)
            a1 = sp.tile([P, TT], F32, tag="a1")
            nc.vector.reduce_sum(a1[:], E4[:, :, 0:2, 2:4], axis=mybir.AxisListType.XY)
            b1 = sp.tile([P, TT], F32, tag="b1")
            nc.vector.reduce_sum(b1[:], E4[:, :, 2:4, 0:2], axis=mybir.AxisListType.XY)
            c1u = sp.tile([P, TT], F32, tag="c1u")
            nc.vector.tensor_tensor(c1u[:], a1[:], b1[:], op=subtract)

            a2 = sp.tile([P, TT], F32, tag="a2")
            nc.vector.reduce_sum(a2[:], E[:, :, 4:8], axis=mybir.AxisListType.X)
            b2 = sp.tile([P, TT], F32, tag="b2")
            nc.vector.reduce_sum(b2[:], E[:, :, 8:12], axis=mybir.AxisListType.X)
            c2u = sp.tile([P, TT], F32, tag="c2u")
            nc.vector.tensor_tensor(c2u[:], a2[:], b2[:], op=subtract)

            # c3 = (E1+E8) + (E11+E13) - (E2+E4) - (E7+E14) - 2*(E6-E9)
            def eg(g):
                return E[:, :, g : g + 1]

            p1 = sp.tile([P, TT, 1], F32, tag="p1")
            nc.vector.tensor_tensor(p1[:], eg(1), eg(8), op=add)
            p2 = sp.tile([P, TT, 1], F32, tag="p2")
            nc.vector.tensor_tensor(p2[:], eg(11), eg(13), op=add)
            n1 = sp.tile([P, TT, 1], F32, tag="n1")
            nc.vector.tensor_tensor(n1[:], eg(2), eg(4), op=add)
            n2 = sp.tile([P, TT, 1], F32, tag="n2")
            nc.vector.tensor_tensor(n2[:], eg(7), eg(14), op=add)
            d6 = sp.tile([P, TT, 1], F32, tag="d6")
            nc.vector.tensor_tensor(d6[:], eg(6), eg(9), op=subtract)
            pp = sp.tile([P, TT, 1], F32, tag="pp")
            nc.vector.tensor_tensor(pp[:], p1[:], p2[:], op=add)
            nn_ = sp.tile([P, TT, 1], F32, tag="nn")
            nc.vector.tensor_tensor(nn_[:], n1[:], n2[:], op=add)
            c3a = sp.tile([P, TT, 1], F32, tag="c3a")
            nc.vector.tensor_tensor(c3a[:], pp[:], nn_[:], op=subtract)
            c3u = sp.tile([P, TT, 1], F32, tag="c3u")
            nc.vector.scalar_tensor_tensor(
                c3u[:], d6[:], -2.0, c3a[:], op0=mult, op1=add
            )

            nc.vector.tensor_tensor(c0[:], c0u[:], r[:], op=mult)
            nc.vector.tensor_tensor(c1[:], c1u[:], r[:], op=mult)
            nc.vector.tensor_tensor(c2[:], c2u[:], r[:], op=mult)
            nc.vector.tensor_tensor(c3[:], c3u[:, :, 0], r[:], op=mult)

        # ---------- main gather + FMA loop ----------
        for ci in range(TT):
            g_t = g_pool.tile([P, 2, batch], F32, tag="g")
            nc.gpsimd.dma_gather(
                g_t[:], xT[:], idx_sb[:, ci * 16 : (ci + 1) * 16], 256, 256, batch
            )
            a_v = g_t[:, 0, :]
            b_v = g_t[:, 1, :]
            cs = slice(ci, ci + 1)
            # u = c0 + c1*a ; v = c2 + c3*a   (ScalarE, per-partition affine)
            u = uv_pool.tile([P, batch], F32, tag="u")
            nc.scalar.activation(u[:], a_v, Ident, bias=c0[:, cs], scale=c1[:, cs])
            v = uv_pool.tile([P, batch], F32, tag="v")
            nc.scalar.activation(v[:], a_v, Ident, bias=c2[:, cs], scale=c3[:, cs])
            # v = v*b + u  (VectorE, in place) then write out
            nc.vector.tensor_tensor(v[:], v[:], b_v, op=mult)
            nc.vector.tensor_tensor(v[:], v[:], u[:], op=add)
            nc.sync.dma_start(outT[ci * P : (ci + 1) * P, :], v[:])

    nc.compile()
    return nc


def _pack_idx(idx_a, idx_b, feat_lo, feat_hi):
    """Host-side int16 gather-index buffer for one core.

    Per 128-feature chunk: 128 idx_a then 128 idx_b. dma_gather consumes
    index i from partition i%16, column i//16 (replicated across the 8
    groups of 16 partitions).
    """
    cols = []
    for f0 in range(feat_lo, feat_hi, P):
        ids = np.concatenate(
            [idx_a[f0 : f0 + P], idx_b[f0 : f0 + P]]
        ).astype(np.int16)
        blk = ids.reshape(16, 16)  # [col, partition-within-16]
        cols.append(np.tile(blk.T, (P // 16, 1)))  # [128, 16]
    return np.ascontiguousarray(np.concatenate(cols, axis=1))


_NC_CACHE = {}


def _get_nc():
    key = (IN_DIM, F_CORE, BATCH)
    if key not in _NC_CACHE:
        _NC_CACHE[key] = _build_nc(IN_DIM, F_CORE, BATCH)
    return _NC_CACHE[key]


TRACE = False  # set by dev harness to capture an NTFF profile
LAST_RESULT = None


def kernel(x, weights, idx_a, idx_b):
    global LAST_RESULT
    from concourse.bass_utils import run_bass_kernel_spmd

    x = np.asarray(x, dtype=np.float32)
    weights = np.asarray(weights, dtype=np.float32)
    idx_a = np.asarray(idx_a)
    idx_b = np.asarray(idx_b)

    nc = _get_nc()
    xT = np.ascontiguousarray(x.T)
    in_maps = []
    for k in range(N_CORES):
        lo, hi = k * F_CORE, (k + 1) * F_CORE
        in_maps.append(
            {
                "xT": xT,
                "w": np.ascontiguousarray(weights[lo:hi]),
                "idx": _pack_idx(idx_a, idx_b, lo, hi),
            }
        )

    res = run_bass_kernel_spmd(nc, in_maps, list(range(N_CORES)), trace=TRACE)
    LAST_RESULT = res
    out = np.empty((BATCH, OUT_DIM), dtype=np.float32)
    for k in range(N_CORES):
        out[:, k * F_CORE : (k + 1) * F_CORE] = res.results[k]["outT"].T
    return out
